# revision 1
# baseline (speedup 1.0000x reference)
"""Trainium2 Bass kernel for nn_MultiHeadedSelfAttention_5179730559275.

Reference math (per batch b):
  q = wq @ x + bq ; k = wk @ x + bk ; v = wv @ x + bv        (1x1 conv, C=256 -> O=256)
  per o-channel (o = head*32 + d), with Q_o,K_o,V_o = 64x64 images [H,W]:
    S_o = Q_o @ K_o^T / sqrt(32); P_o = softmax(S_o, axis=-1); ctx_o = P_o @ V_o

Sharding: data-parallel over batch, 2 batches per core on 8 cores.

Per-core pipeline (per batch):
  1. fp32 x tiles [c, pix] -> PE projections (lhsT = w^T fp16 stationary,
     rhs = x as float32r moving, N=512) -> psum [o', 512]
  2. psum->SBUF copies add bias, cast fp16, and write interleaved layouts
     pairing o with o+128 (om = o chunk):
       q16/k16: [j, h*128 + om*64 + w]   (j = o mod 128)
       v16:     [j, w*128 + om*64 + g]
  3. xbar DMA transposes of 128x128 slices give matmul-ready layouts:
       qS/kS: [om*64 + w, h, j]  (per-o transposed images, o-pair stacked)
       vS:    [om*64 + g, w, j]  (natural images + ones column for Z)
  4. Attention per pair j: quadrant matmuls (K=64 at partition bases 0/64):
       S^T psum [om*64+g, h] ; exp (ACT, bias -2) -> eS fp16
       ctx psum [om*64+h, 0:64]=E^T.T@V, col 64 = Z (ones column)
     normalize on DVE: ctx = psum * (1/Z) + bv, DMA out.
"""

import numpy as np

import concourse.bass as bass
import concourse.bacc as bacc
import concourse.tile as tile
from concourse import mybir
from concourse import bass2jax

NCORES = 8
B, C, H, W = 16, 256, 64, 64
O = 256
PIX = H * W
BL = B // NCORES  # batches per core
SCALE = 1.0 / float(np.sqrt(32.0))
EXP_BIAS = -2.0  # softmax-invariant shift keeping exp() well inside fp16 range

FP32 = mybir.dt.float32
FP32R = mybir.dt.float32r
FP16 = mybir.dt.float16


def build_kernel(nc: bass.Bass):
    x_in = nc.declare_dram_parameter("x", [BL, C, PIX], FP16, isOutput=False)
    wT_in = nc.declare_dram_parameter("wT", [3, C, O], FP16, isOutput=False)
    bias_in = nc.declare_dram_parameter("bias", [3, O], FP32, isOutput=False)
    bv_in = nc.declare_dram_parameter("bv", [O], FP32, isOutput=False)
    out = nc.declare_dram_parameter("out", [BL, O, PIX], FP16, isOutput=True)

    with tile.TileContext(nc) as tc:
        with (
            tc.tile_pool(name="singles", bufs=1) as singles,
            tc.tile_pool(name="xin", bufs=2) as xpool,
            tc.tile_pool(name="p16", bufs=1) as p16pool,
            tc.tile_pool(name="tsp", bufs=2) as tpool,
            tc.tile_pool(name="small", bufs=8) as small,
            tc.tile_pool(name="psA", bufs=3, space="PSUM") as psA,
            tc.tile_pool(name="psS", bufs=2, space="PSUM") as psS,
            tc.tile_pool(name="psC", bufs=3, space="PSUM") as psC,
        ):
            # ---- constants loaded once ----
            w_sb = singles.tile([128, 3, 2, O], FP16)  # [c', proj, cc, o]
            nc.sync.dma_start(
                out=w_sb,
                in_=wT_in.rearrange("t (cc c) o -> c t cc o", cc=2),
            )
            bias_sb = singles.tile([128, 3, 2], FP32)  # [o', proj, oc]
            nc.sync.dma_start(
                out=bias_sb,
                in_=bias_in.rearrange("t (oc o) -> o t oc", oc=2),
            )
            # bv broadcast to [p=(om,h), j]
            bv_sb = singles.tile([128, 128], FP32)
            bv_ap = bv_in[:]
            bv_bcast = bass.AP(
                tensor=bv_ap.tensor,
                offset=bv_ap.offset,
                ap=[[128, 2], [0, 64], [1, 128]],
            )
            nc.sync.dma_start(out=bv_sb, in_=bv_bcast)
            expb_sb = singles.tile([128, 1], FP32)
            nc.vector.memset(expb_sb, EXP_BIAS)

            tensors = {}

            def emit_front(b):
                xsb = []
                for cc in range(2):
                    xt = xpool.tile([128, PIX], FP16, tag="xsb")
                    nc.sync.dma_start(out=xt, in_=x_in[b, cc * 128 : (cc + 1) * 128, :])
                    xsb.append(xt)

                q16 = p16pool.tile([128, H, 2, W], FP16, tag="q16")  # [j, h, om, w]
                k16 = p16pool.tile([128, H, 2, W], FP16, tag="k16")
                v16 = p16pool.tile([128, W, 2, H], FP16, tag="v16")  # [j, w, om, g]

                for proj in range(3):
                    for oc in range(2):
                        for nt in range(8):
                            ps = psA.tile([128, 512], FP32, tag="ps_proj")
                            for cc in range(2):
                                nc.tensor.matmul(
                                    ps,
                                    lhsT=w_sb[:, proj, cc, oc * 128 : (oc + 1) * 128],
                                    rhs=xsb[cc][:, nt * 512 : (nt + 1) * 512],
                                    start=(cc == 0),
                                    stop=(cc == 1),
                                )
                            bias_ap = bias_sb[:, proj, oc : oc + 1]
                            if proj == 0:  # q
                                nc.scalar.activation(
                                    out=q16[:, nt * 8 : (nt + 1) * 8, oc, :],
                                    in_=ps.rearrange("p (h w) -> p h w", w=W),
                                    func=mybir.ActivationFunctionType.Identity,
                                    bias=bias_ap,
                                    scale=1.0,
                                )
                            elif proj == 1:  # k
                                nc.vector.tensor_scalar_add(
                                    out=k16[:, nt * 8 : (nt + 1) * 8, oc, :],
                                    in0=ps.rearrange("p (h w) -> p h w", w=W),
                                    scalar1=bias_ap,
                                )
                            else:  # v
                                nc.vector.tensor_scalar_add(
                                    out=v16[:, :, oc, nt * 8 : (nt + 1) * 8].rearrange(
                                        "p w g -> p g w"
                                    ),
                                    in0=ps.rearrange("p (g w) -> p g w", w=W),
                                    scalar1=bias_ap,
                                )

                qS = tpool.tile([128, H, 128], FP16, tag="qS")  # [om*64+w, h, j]
                kS = tpool.tile([128, H, 128], FP16, tag="kS")
                vS = tpool.tile([128, W + 1, 128], FP16, tag="vS")
                nc.vector.memset(vS[:, W, :], 1.0)
                for h in range(H):
                    nc.sync.dma_start_transpose(
                        out=kS[:, h, :], in_=k16[:, h, :, :].rearrange("p a b -> p (a b)")
                    )
                    nc.sync.dma_start_transpose(
                        out=qS[:, h, :], in_=q16[:, h, :, :].rearrange("p a b -> p (a b)")
                    )
                for h in range(H):
                    nc.sync.dma_start_transpose(
                        out=vS[:, h, :], in_=v16[:, h, :, :].rearrange("p a b -> p (a b)")
                    )
                tensors[b] = (qS, kS, vS)

            def emit_attn(b):
                qS, kS, vS = tensors[b]
                JG = 8
                PG = 4
                for jg in range(0, 128, JG):
                    oc8 = small.tile([128, JG, W], FP16, tag="oc8")
                    sp8f = psS.tile([128, 512], FP32, tag="sp8")
                    sp8 = sp8f.rearrange("p (i h) -> p i h", h=H)
                    for i in range(JG):
                        j = jg + i
                        for om in range(2):
                            pr = slice(om * 64, om * 64 + 64)
                            nc.tensor.matmul(
                                sp8[pr, i, :],
                                lhsT=kS[pr, :, j],
                                rhs=qS[pr, :, j],
                                start=True,
                                stop=True,
                            )
                    eS8 = small.tile([128, JG, H], FP16, tag="eS8")
                    nc.scalar.activation(
                        out=eS8,
                        in_=sp8,
                        func=mybir.ActivationFunctionType.Exp,
                        bias=expb_sb,
                        scale=1.0,
                    )
                    for sg in range(jg, jg + JG, PG):
                        cp4f = psC.tile([128, 512], FP32, tag="cp4")
                        cp4 = cp4f[:, 0 : PG * (W + 1)].rearrange(
                            "p (i c) -> p i c", c=W + 1
                        )
                        for i in range(PG):
                            j = sg + i
                            for om in range(2):
                                pr = slice(om * 64, om * 64 + 64)
                                nc.tensor.matmul(
                                    cp4[pr, i, :],
                                    lhsT=eS8[pr, j - jg, :],
                                    rhs=vS[pr, :, j],
                                    start=True,
                                    stop=True,
                                )
                        rz4 = small.tile([128, PG], FP32, tag="rz4")
                        nc.vector.reciprocal(out=rz4, in_=cp4[:, :, W])
                        for i in range(PG):
                            j = sg + i
                            if j % 2 == 0:
                                nc.scalar.activation(
                                    out=oc8[:, j - jg, :],
                                    in_=cp4[:, i, 0:W],
                                    func=mybir.ActivationFunctionType.Identity,
                                    bias=bv_sb[:, j : j + 1],
                                    scale=rz4[:, i : i + 1],
                                )
                            else:
                                nc.vector.tensor_scalar(
                                    out=oc8[:, j - jg, :],
                                    in0=cp4[:, i, 0:W],
                                    scalar1=rz4[:, i : i + 1],
                                    scalar2=bv_sb[:, j : j + 1],
                                    op0=mybir.AluOpType.mult,
                                    op1=mybir.AluOpType.add,
                                )
                    for om in range(2):
                        nc.sync.dma_start(
                            out=out[b, om * 128 + jg : om * 128 + jg + JG, :].rearrange(
                                "j (h w) -> h j w", w=W
                            ),
                            in_=oc8[om * 64 : om * 64 + 64, :, :],
                        )

            for b in range(BL):
                emit_front(b)
            for b in range(BL):
                emit_attn(b)
    return nc


_NC_CACHE = {}


def get_nc():
    if "nc" not in _NC_CACHE:
        nc = bacc.Bacc(None, target_bir_lowering=False)
        build_kernel(nc)
        nc.finalize()
        _NC_CACHE["nc"] = nc
    return _NC_CACHE["nc"]


def prep_in_maps(x, wq, bq, wk, bk, wv, bv):
    wT = np.stack(
        [
            np.ascontiguousarray((wq * SCALE).T),
            np.ascontiguousarray(wk.T),
            np.ascontiguousarray(wv.T),
        ]
    ).astype(np.float16)
    biases = np.stack([bq * SCALE, bk, np.zeros_like(bv)]).astype(np.float32)
    # note: bv is folded in at the output stage, not in the v projection
    xs = np.ascontiguousarray(x.reshape(NCORES, BL, C, PIX)).astype(np.float16)
    bv32 = np.ascontiguousarray(bv).astype(np.float32)
    return [
        {"x": xs[i], "wT": wT, "bias": biases, "bv": bv32} for i in range(NCORES)
    ]


def kernel(x, wq, bq, wk, bk, wv, bv):
    nc = get_nc()
    in_maps = prep_in_maps(x, wq, bq, wk, bk, wv, bv)
    results = bass2jax.run_bass_via_pjrt(nc, in_maps, n_cores=NCORES)
    outs = [np.asarray(r["out"]).reshape(BL, O, H, W) for r in results]
    return np.concatenate(outs, axis=0).astype(np.float32)



# revision 17
# speedup vs baseline: 3.3488x; 3.3488x over previous
"""Trainium2 Bass kernel for nn_MultiHeadedSelfAttention_5179730559275.

Reference math (per batch b):
  q = wq @ x + bq ; k = wk @ x + bk ; v = wv @ x + bv        (1x1 conv, C=256 -> O=256)
  per o-channel (o = head*32 + d), with Q_o,K_o,V_o = 64x64 images [H,W]:
    S_o = Q_o @ K_o^T / sqrt(32); P_o = softmax(S_o, axis=-1); ctx_o = P_o @ V_o

Sharding: data-parallel over batch, 2 batches per core on 8 cores.

Per-core pipeline (per batch):
  1. fp16 projections on PE (lhsT = w^T stationary, rhs = x moving, N=512)
     -> psum [o', 512]; psum->SBUF copies add bias + cast fp16 into
     interleaved layouts pairing o with o+128 (om):
       q16/k16/v16: [j, r, om, c]  (j = o mod 128; r,c = h,w image coords)
  2. PE transposes (is_transpose matmul vs fp16 identity) flip 128x128
     slices [j, (om,c)] -> [(om,c), j], 8 per psum bank, then one batched
     copy per bank gives matmul-ready layouts:
       qS/kS: [om*64 + w, h, j] ; vS: [om*64 + g, w, j] (+ ones col for Z)
     (vS transposes read v16 with a strided weights AP to swap (g,w).)
  3. Attention per pair j: quadrant matmuls (K=64 at partition bases 0/64):
       S^T psum [om*64+g, h] ; exp (ACT, bias -2) -> eS fp16
       ctx psum [om*64+h, 0:64]=E^T.T@V, col 64 = Z (ones column)
     normalize: ctx = psum * (1/Z) + bv, DMA out.

Engine split: q-copies on ACT, k on DVE, v on Pool; transpose-bank copies
round-robin ACT/DVE/Pool so PE (the bottleneck) never waits.
"""

import numpy as np

import concourse.bass as bass
import concourse.bacc as bacc
import concourse.tile as tile
from concourse import mybir, masks
from concourse import bass2jax

NCORES = 8
B, C, H, W = 16, 256, 64, 64
O = 256
PIX = H * W
BL = B // NCORES  # batches per core
SCALE = 1.0 / float(np.sqrt(32.0))
EXP_BIAS = -2.0  # softmax-invariant shift keeping exp() well inside fp16 range

FP32 = mybir.dt.float32
FP16 = mybir.dt.float16


def build_kernel(nc: bass.Bass):
    x_in = nc.declare_dram_parameter("x", [BL, C, PIX], FP16, isOutput=False)
    # x with each 64x64 image transposed (w-major pixels); feeds the V
    # projection so V's psum comes out w-major and every transpose input
    # is contiguous.
    xt_in = nc.declare_dram_parameter("xt", [BL, C, PIX], FP16, isOutput=False)
    wT_in = nc.declare_dram_parameter("wT", [3, C, O], FP16, isOutput=False)
    bias_in = nc.declare_dram_parameter("bias", [3, O], FP32, isOutput=False)
    out = nc.declare_dram_parameter("out", [BL, O, PIX], FP16, isOutput=True)

    with tile.TileContext(nc) as tc:
        with (
            tc.tile_pool(name="singles", bufs=1) as singles,
            tc.tile_pool(name="xin", bufs=2) as xpool,
            tc.tile_pool(name="p16", bufs=1) as p16pool,
            tc.tile_pool(name="tsp", bufs=2) as tpool,
            tc.tile_pool(name="small", bufs=8) as small,
            tc.tile_pool(name="psA", bufs=2, space="PSUM") as psA,
            tc.tile_pool(name="psT", bufs=2, space="PSUM") as psT,
            tc.tile_pool(name="psS", bufs=2, space="PSUM") as psS,
            tc.tile_pool(name="psC", bufs=2, space="PSUM") as psC,
        ):
            # ---- constants loaded once ----
            w_sb = singles.tile([128, 3, 2, O], FP16)  # [c', proj, cc, o]
            nc.sync.dma_start(
                out=w_sb,
                in_=wT_in.rearrange("t (cc c) o -> c t cc o", cc=2),
            )
            bias_sb = singles.tile([128, 3, 2], FP32)  # [o', proj, oc]
            nc.sync.dma_start(
                out=bias_sb,
                in_=bias_in.rearrange("t (oc o) -> o t oc", oc=2),
            )
            expb_sb = singles.tile([128, 1], FP32)
            nc.vector.memset(expb_sb, EXP_BIAS)
            ident = singles.tile([128, 128], FP16)
            masks.make_identity(nc, ident[:])

            tensors = {}
            copy_fns = [
                lambda o, i: nc.scalar.copy(o, i),
                lambda o, i: nc.vector.tensor_copy(o, i),
            ]

            def emit_front(b):
                xsb, xtsb = [], []
                for cc in range(2):
                    xt = xpool.tile([128, PIX], FP16, tag="xsb")
                    xtt = xpool.tile([128, PIX], FP16, tag="xtb")
                    for piece in range(4):
                        sl = slice(piece * (PIX // 4), (piece + 1) * (PIX // 4))
                        nc.sync.dma_start(
                            out=xt[:, sl], in_=x_in[b, cc * 128 : (cc + 1) * 128, sl]
                        )
                        nc.sync.dma_start(
                            out=xtt[:, sl], in_=xt_in[b, cc * 128 : (cc + 1) * 128, sl]
                        )
                    xsb.append(xt)
                    xtsb.append(xtt)

                # [j, r, om, c]: r,c are image coords (h,w); j = o mod 128
                q16 = p16pool.tile([128, H, 2, W], FP16, tag="q16")
                k16 = p16pool.tile([128, H, 2, W], FP16, tag="k16")
                v16 = p16pool.tile([128, H, 2, W], FP16, tag="v16")
                p16 = [q16, k16, v16]

                # nt-outer, proj-inner so consecutive psum drains alternate
                # ACT/DVE and the PE never waits on one engine. (Pool/GpSimd
                # cannot access PSUM on TRN2.)
                for nt in range(8):
                    for proj in range(3):
                        for oc in range(2):
                            ps = psA.tile([128, 512], FP32, tag="ps_proj")
                            xin = xtsb if proj == 2 else xsb
                            for cc in range(2):
                                nc.tensor.matmul(
                                    ps,
                                    lhsT=w_sb[:, proj, cc, oc * 128 : (oc + 1) * 128],
                                    rhs=xin[cc][:, nt * 512 : (nt + 1) * 512],
                                    start=(cc == 0),
                                    stop=(cc == 1),
                                )
                            bias_ap = bias_sb[:, proj, oc : oc + 1]
                            dst = p16[proj][:, nt * 8 : (nt + 1) * 8, oc, :]
                            src = ps.rearrange("p (r c) -> p r c", c=W)
                            if (proj * 2 + oc) % 2 == 0:
                                nc.scalar.activation(
                                    out=dst,
                                    in_=src,
                                    func=mybir.ActivationFunctionType.Identity,
                                    bias=bias_ap,
                                    scale=1.0,
                                )
                            else:
                                nc.vector.tensor_scalar_add(
                                    out=dst, in0=src, scalar1=bias_ap
                                )

                qS = tpool.tile([128, H, 128], FP16, tag="qS")  # [om*64+w, h, j]
                kS = tpool.tile([128, H, 128], FP16, tag="kS")
                vS = tpool.tile([128, W + 1, 128], FP16, tag="vS")  # [om*64+g, w, j]
                nc.vector.memset(vS[:, W, :], 1.0)

                # PE transposes: 8 per psum bank, then one batched copy/bank.
                # q/k sliced at fixed h give [j, (om, w)]; v16 is w-major
                # (projected from xt) so its slices give [j, (om, g)].
                eng_i = 0
                for t16, tS in ((q16, qS), (k16, kS), (v16, vS)):
                    for t0 in range(0, H, 8):
                        pt = psT.tile([128, 8, 128], FP16, tag="pst")
                        for i in range(8):
                            lhsT = t16[:, t0 + i, :, :].rearrange(
                                "p om w -> p (om w)"
                            )
                            nc.tensor.transpose(pt[:, i, :], lhsT, ident)
                        copy_fns[eng_i % 2](tS[:, t0 : t0 + 8, :], pt)
                        eng_i += 1
                tensors[b] = (qS, kS, vS)

            def emit_attn(b):
                qS, kS, vS = tensors[b]
                JG = 8
                PG = 4
                for jg in range(0, 128, JG):
                    oc8 = small.tile([128, JG, W], FP16, tag="oc8")
                    sp8f = psS.tile([128, 512], FP32, tag="sp8")
                    sp8 = sp8f.rearrange("p (i h) -> p i h", h=H)
                    for i in range(JG):
                        j = jg + i
                        for om in range(2):
                            pr = slice(om * 64, om * 64 + 64)
                            nc.tensor.matmul(
                                sp8[pr, i, :],
                                lhsT=kS[pr, :, j],
                                rhs=qS[pr, :, j],
                                start=True,
                                stop=True,
                            )
                    eS8 = small.tile([128, JG, H], FP16, tag="eS8")
                    nc.scalar.activation(
                        out=eS8,
                        in_=sp8,
                        func=mybir.ActivationFunctionType.Exp,
                        bias=expb_sb,
                        scale=1.0,
                    )
                    for sg in range(jg, jg + JG, PG):
                        cp4f = psC.tile([128, 512], FP32, tag="cp4")
                        cp4 = cp4f[:, 0 : PG * (W + 1)].rearrange(
                            "p (i c) -> p i c", c=W + 1
                        )
                        for i in range(PG):
                            j = sg + i
                            for om in range(2):
                                pr = slice(om * 64, om * 64 + 64)
                                nc.tensor.matmul(
                                    cp4[pr, i, :],
                                    lhsT=eS8[pr, j - jg, :],
                                    rhs=vS[pr, :, j],
                                    start=True,
                                    stop=True,
                                )
                        rz4 = small.tile([128, PG], FP32, tag="rz4")
                        nc.vector.reciprocal(out=rz4, in_=cp4[:, :, W])
                        # bv is folded into the V projection bias, so the
                        # normalize is a single broadcast multiply per group.
                        rzf = rz4[:]
                        rzb = bass.AP(
                            tensor=rzf.tensor,
                            offset=rzf.offset,
                            ap=[rzf.ap[0], rzf.ap[1], [0, W]],
                        )
                        nc.vector.tensor_mul(
                            out=oc8[:, sg - jg : sg - jg + PG, :],
                            in0=cp4[:, :, 0:W],
                            in1=rzb,
                        )
                    for om in range(2):
                        nc.sync.dma_start(
                            out=out[b, om * 128 + jg : om * 128 + jg + JG, :].rearrange(
                                "j (h w) -> h j w", w=W
                            ),
                            in_=oc8[om * 64 : om * 64 + 64, :, :],
                        )

            for b in range(BL):
                emit_front(b)
            for b in range(BL):
                emit_attn(b)
    return nc


_NC_CACHE = {}


def get_nc():
    if "nc" not in _NC_CACHE:
        nc = bacc.Bacc(None, target_bir_lowering=False)
        build_kernel(nc)
        nc.finalize()
        _NC_CACHE["nc"] = nc
    return _NC_CACHE["nc"]


def prep_in_maps(x, wq, bq, wk, bk, wv, bv):
    wT = np.stack(
        [
            np.ascontiguousarray((wq * SCALE).T),
            np.ascontiguousarray(wk.T),
            np.ascontiguousarray(wv.T),
        ]
    ).astype(np.float16)
    # bv is folded into the V projection bias: softmax weights sum to 1, so
    # (sum_g P*(V+bv)) == (sum_g P*V) + bv exactly.
    biases = np.stack([bq * SCALE, bk, bv]).astype(np.float32)
    x16 = x.astype(np.float16)
    xs = np.ascontiguousarray(x16.reshape(NCORES, BL, C, PIX))
    xts = np.ascontiguousarray(
        x16.reshape(NCORES, BL, C, H, W).transpose(0, 1, 2, 4, 3)
    ).reshape(NCORES, BL, C, PIX)
    return [
        {"x": xs[i], "xt": xts[i], "wT": wT, "bias": biases} for i in range(NCORES)
    ]


def kernel(x, wq, bq, wk, bk, wv, bv):
    nc = get_nc()
    in_maps = prep_in_maps(x, wq, bq, wk, bk, wv, bv)
    results = bass2jax.run_bass_via_pjrt(nc, in_maps, n_cores=NCORES)
    outs = [np.asarray(r["out"]).reshape(BL, O, H, W) for r in results]
    return np.concatenate(outs, axis=0).astype(np.float32)
